# revision 34
# baseline (speedup 1.0000x reference)
"""GAT NodeEncoder kernel for Trainium2 (8 NeuronCores, data-parallel over batch).

Reference computation (per batch element b, per node n):
    src  = E[subgraph[b,n]];  nei_i = E[neighs[b,n,i]]
    s_0  = leaky(src@a1 + src@a2 + a_b); s_i = leaky(src@a1 + nei_i@a2 + a_b) + mask_i*-1e9
    att  = softmax(s); v = sum_i att_i * emb_i
    x = leaky(fc1 @ [v; local_stats; gstat] + b1); out = leaky(fc2 @ x + b2)

Sharding: batch B=8 over 8 cores (1 batch row / core), emb table replicated
(uploaded bf16 -- matches the bf16 in-flight compute precision).

Gather strategy. SWDGE descriptor generation costs ~8ns per gathered row on
one Q7 pair, but the 4 SWDGE queues run on different Q7 pairs and overlap
when calls are issued back-to-back. Two-phase scheme, split into halves so
compute starts early:
  Phase 1 (per tile-group of 4): gather the group's UNIQUE table rows with
  dma_gather calls per 32768-row window (int16 idx limit), chunked
  round-robin over the 4 queues; flush each chunk to a DRAM scratch region
  addressable by int16 rank.
  Phase 2 (per group): per 128-node tile, 4 dma_gather sub-calls from the
  group's scratch fetch all (node, slot) rows positionally (slot-major).
  Queue numbers are rewritten post-schedule to match the Tile-assigned
  DMASW semaphore lanes (lane %% 4).

Compute per tile (slots uniform per tile via degree-sorting, masked
neighbors dropped exactly):
  scores   = reduce_X(g * a2_bcast) on DVE; s = leaky(w + u + ab) ACT+DVE
  softmax  = max / exp+accum / recip; att = Copy(e, scale=1/z) on ACT
  weighted = g * att (stride-0-inner broadcast) + halving add-tree on DVE
  head     = PE transpose via identity, fc1/fc2 on PE, bias on ACT
Output rows are stored directly (HWDGE) in sorted order; host unsorts.
"""

import os
from contextlib import ExitStack

import numpy as np
import ml_dtypes

import concourse.bass as bass
import concourse.bacc as bacc
import concourse.tile as tile
from concourse import mybir
from concourse import bass_utils
from concourse import library_config

B, S, N, H, NLS = 8, 1024, 32, 128, 4
NUM_NODES = 100001
TILE = 128
NT = S // TILE
WIN = 32768
NWIN = 4
GROUPS = ((0, 1, 2, 3, 4), (5, 6, 7))      # tile groups (phase-1 split)
NG = len(GROUPS)
F32 = mybir.dt.float32
BF16 = mybir.dt.bfloat16
I32 = mybir.dt.int32
I16 = mybir.dt.int16
AF = mybir.ActivationFunctionType
ALU = mybir.AluOpType

_cached = {}


def _rup(x, m):
    return (x + m - 1) // m * m


def _win_chunks(niw):
    """Split a window's padded index list into <=4 chunks at 128-pos bounds."""
    if niw < 1024:
        return [(0, niw)]
    nch = 8 if niw >= 4096 else 4
    q = _rup((niw + nch - 1) // nch, TILE)
    return [(p0, min(q, niw - p0)) for p0 in range(0, niw, q)]


def _build_program(slots, ni_hw, v_hw):
    """slots: per-tile slot counts (len 8); ni_hw/v_hw: per-GROUP tuples of
    per-window phase-1 static num_idxs / valid counts (SPMD-uniform)."""
    nt = len(slots)
    ctot = int(sum(slots))
    offs = np.concatenate([[0], np.cumsum(slots)]).astype(int)
    cmax = int(max(slots))

    # per-group scratch geometry
    blocks_hw = [[_rup(ni, TILE) // TILE for ni in ni_hw[h]] for h in range(NG)]
    blk_off_h = [np.concatenate([[0], np.cumsum(blocks_hw[h])]).astype(int)
                 for h in range(NG)]
    blk_tot_h = [int(blk_off_h[h][-1]) for h in range(NG)]
    srows_h = [TILE * bt for bt in blk_tot_h]
    sbase_h = np.concatenate([[0], np.cumsum(srows_h)]).astype(int)
    srows = int(sum(srows_h))
    # phase-1 idx columns per (group, window)
    n1cols_h = [[ni // 16 for ni in ni_hw[h]] for h in range(NG)]
    c1off_h = [np.concatenate([[0], np.cumsum(n1cols_h[h])]).astype(int)
               for h in range(NG)]
    c1base = np.concatenate([[0], np.cumsum([int(c1off_h[h][-1]) for h in range(NG)])]).astype(int)
    c1tot = int(c1base[-1])
    # phase-2 idx columns per tile
    n2cols = [TILE * int(c) // 16 for c in slots]
    c2off = np.concatenate([[0], np.cumsum(n2cols)]).astype(int)

    nc = bacc.Bacc(target_bir_lowering=False, debug=False, enable_asserts=False,
                   num_swdge_queues=4)

    emb = nc.dram_tensor("emb", [NUM_NODES, H], BF16, kind="ExternalInput")
    idx1 = nc.dram_tensor("idx1", [TILE, c1tot], I16, kind="ExternalInput")
    idx2 = nc.dram_tensor("idx2", [TILE, int(c2off[-1])], I16, kind="ExternalInput")
    padm = nc.dram_tensor("padm", [TILE, ctot], F32, kind="ExternalInput")
    statst = nc.dram_tensor("statst", [NLS + 1, S], BF16, kind="ExternalInput")
    a2rep_d = nc.dram_tensor("a2rep", [1, H], BF16, kind="ExternalInput")
    a1rep_d = nc.dram_tensor("a1rep", [1, H], BF16, kind="ExternalInput")
    ab_rep = nc.dram_tensor("ab_rep", [TILE, 1], F32, kind="ExternalInput")
    ident = nc.dram_tensor("ident", [TILE, TILE], BF16, kind="ExternalInput")
    w1t_a = nc.dram_tensor("w1t_a", [H, H], BF16, kind="ExternalInput")
    w1t_b = nc.dram_tensor("w1t_b", [NLS + 1, H], BF16, kind="ExternalInput")
    b1 = nc.dram_tensor("b1", [H, 1], F32, kind="ExternalInput")
    w2t = nc.dram_tensor("w2t", [H, H], BF16, kind="ExternalInput")
    b2row = nc.dram_tensor("b2row", [1, H], BF16, kind="ExternalInput")
    onesc = nc.dram_tensor("onesc", [1, TILE], BF16, kind="ExternalInput")
    out = nc.dram_tensor("out", [S, H], F32, kind="ExternalOutput")

    with tile.TileContext(nc) as tc, ExitStack() as ctx:
        dpool = ctx.enter_context(tc.tile_pool(name="dram", bufs=1, space="DRAM"))
        const = ctx.enter_context(tc.tile_pool(name="const", bufs=1))
        psum = ctx.enter_context(tc.tile_pool(name="psum", bufs=2, space="PSUM"))

        scratch = dpool.tile([srows, H], BF16)

        nc.gpsimd.load_library(library_config.mlp)

        # ---- constants: idx1 first (gates phase-1), all on the sync queue
        # (idle until flushes) so the ACT engine is free for early compute ----
        p1pool = ctx.enter_context(tc.tile_pool(name="p1", bufs=1))
        c_idx1 = p1pool.tile([TILE, c1tot], I16)
        nc.sync.dma_start(out=c_idx1[:], in_=idx1[:, :])
        c_idx2_0 = const.tile([TILE, int(c2off[-1])], I16)
        nc.sync.dma_start(out=c_idx2_0[:], in_=idx2[:, :])
        c_padm0 = const.tile([TILE, ctot], F32)
        nc.sync.dma_start(out=c_padm0[:], in_=padm[:, :])
        c_stats = const.tile([NLS + 1, S], BF16)
        nc.sync.dma_start(out=c_stats[:], in_=statst[:, :])
        c_ab = const.tile([TILE, 1], F32)
        nc.sync.dma_start(out=c_ab[:], in_=ab_rep[:, :])
        c_id = const.tile([TILE, TILE], BF16)
        nc.sync.dma_start(out=c_id[:], in_=ident[:, :])
        c_w1a = const.tile([H, H], BF16)
        nc.sync.dma_start(out=c_w1a[:], in_=w1t_a[:, :])
        c_w1b = const.tile([NLS + 1, H], BF16)
        nc.sync.dma_start(out=c_w1b[:], in_=w1t_b[:, :])
        c_b1 = const.tile([H, 1], F32)
        nc.sync.dma_start(out=c_b1[:], in_=b1[:, :])
        c_w2 = const.tile([H, H], BF16)
        nc.sync.dma_start(out=c_w2[:], in_=w2t[:, :])
        c_b2 = const.tile([1, H], BF16)
        nc.sync.dma_start(out=c_b2[:], in_=b2row[:, :])
        c_ones = const.tile([1, TILE], BF16)
        nc.sync.dma_start(out=c_ones[:], in_=onesc[:, :])
        # a1/a2 rows replicated to 128 partitions via PE ones-trick (keeps the
        # Pool engine free of non-gather DMAs so DMASW lanes map 1:1 to queues)
        c_a2row = const.tile([1, H], BF16)
        nc.sync.dma_start(out=c_a2row[:], in_=a2rep_d[:, :])
        c_a1row = const.tile([1, H], BF16)
        nc.sync.dma_start(out=c_a1row[:], in_=a1rep_d[:, :])

        # ---- fences: absorb const-DMA sems onto consuming engines ----
        c_idx2 = const.tile([TILE, int(c2off[-1])], I16)
        nc.vector.tensor_copy(out=c_idx2[:], in_=c_idx2_0[:])
        reps = psum.tile([TILE, H], F32, tag="dfence")
        c_a2r = const.tile([TILE, H], BF16)
        nc.tensor.matmul(out=reps[:], lhsT=c_ones[:], rhs=c_a2row[:], start=True, stop=True)
        nc.scalar.activation(out=c_a2r[:], in_=reps[:], func=AF.Copy)
        reps2 = psum.tile([TILE, H], F32, tag="dfence")
        c_a1r = const.tile([TILE, H], BF16)
        nc.tensor.matmul(out=reps2[:], lhsT=c_ones[:], rhs=c_a1row[:], start=True, stop=True)
        nc.scalar.activation(out=c_a1r[:], in_=reps2[:], func=AF.Copy)
        c_padm = const.tile([TILE, ctot], F32)
        nc.vector.tensor_copy(out=c_padm[:], in_=c_padm0[:])
        c_ab2 = const.tile([TILE, 1], F32)
        nc.vector.tensor_copy(out=c_ab2[:], in_=c_ab[:])
        dpsum = psum.tile([TILE, TILE], F32, tag="dfence")
        nc.tensor.matmul(out=dpsum[:], lhsT=c_id[:], rhs=c_w1a[:], start=True, stop=True)
        nc.tensor.matmul(out=dpsum[:], lhsT=c_w2[:], rhs=c_id[:], start=True, stop=True)
        nc.tensor.matmul(
            out=dpsum[:], lhsT=c_w1b[:], rhs=c_stats[:, 0:TILE], start=True, stop=True)
        nc.tensor.matmul(out=dpsum[:], lhsT=c_ones[:], rhs=c_b2[:], start=True, stop=True)
        dact = const.tile([TILE, 1], F32)
        nc.scalar.activation(out=dact[:], in_=c_ab2[:], func=AF.Identity, bias=c_b1[:, 0:1])

        gpool = ctx.enter_context(tc.tile_pool(name="gpool", bufs=1))
        spool = ctx.enter_context(tc.tile_pool(name="spool", bufs=3))
        small = ctx.enter_context(tc.tile_pool(name="small", bufs=6))
        opool = ctx.enter_context(tc.tile_pool(name="opool", bufs=2))

        qrr = 0
        gtiles = {}

        def phase1(h):
            nonlocal qrr
            g1 = p1pool.tile([TILE, blk_tot_h[h] * H], BF16, tag=f"g1{h}")
            blk_off = blk_off_h[h]
            for w in range(NWIN):
                niw, vw = int(ni_hw[h][w]), int(v_hw[h][w])
                if niw == 0:
                    continue
                span = min(WIN, NUM_NODES - w * WIN)
                src_ap = bass.AP(tensor=emb.ap().tensor, offset=w * WIN * H,
                                 ap=[[H, span], [1, H]])
                for (p0, ln) in _win_chunks(niw):
                    vc = max(0, min(ln, vw - p0))
                    b0 = int(blk_off[w]) + p0 // TILE
                    nblk = _rup(ln, TILE) // TILE
                    nc.gpsimd.dma_gather(
                        g1[:, b0 * H:(b0 + nblk) * H].rearrange(
                            "p (b h) -> p b h", b=nblk),
                        src_ap,
                        c_idx1[:, int(c1base[h]) + int(c1off_h[h][w]) + p0 // 16:
                               int(c1base[h]) + int(c1off_h[h][w]) + (p0 + ln) // 16],
                        ln, vc, H,
                        single_packet=False, queue_num=qrr % 4)
                    qrr += 1
                    # flush chunk to scratch rows sbase + p*blk_tot + b0 + u
                    nc.sync.dma_start(
                        out=bass.AP(
                            tensor=scratch[:].tensor,
                            offset=scratch[:].offset + (int(sbase_h[h]) + b0) * H,
                            ap=[[blk_tot_h[h] * H, TILE], [H, nblk], [1, H]]),
                        in_=g1[:, b0 * H:(b0 + nblk) * H])

        def phase2(h):
            nonlocal qrr
            for t in GROUPS[h]:
                ct = int(slots[t])
                g = gpool.tile([TILE, ct * H], BF16, tag=f"g{t}")
                src = bass.AP(tensor=scratch[:].tensor,
                              offset=scratch[:].offset + int(sbase_h[h]) * H,
                              ap=[[H, srows_h[h]], [1, H]])
                cq = (ct + 3) // 4
                bounds = list(range(0, ct, cq)) + [ct]
                for (s0_, s1_) in zip(bounds[:-1], bounds[1:]):
                    nidx = TILE * (s1_ - s0_)
                    nc.gpsimd.dma_gather(
                        g[:, s0_ * H:s1_ * H].rearrange(
                            "p (i h) -> p i h", i=s1_ - s0_),
                        src,
                        c_idx2[:, int(c2off[t]) + s0_ * 8:
                               int(c2off[t]) + s1_ * 8],
                        nidx, nidx, H,
                        single_packet=False, queue_num=qrr % 4)
                    qrr += 1
                gtiles[t] = g

        for h in range(NG):
            phase1(h)
            phase2(h)

        # ---- per-tile compute (software-pipelined: tile t+1's score pass is
        # emitted between tile t's softmax and weighted-sum so DVE has work
        # while ACT runs) ----
        def scores(t):
            ct = int(slots[t])
            g = gtiles[t]
            t1 = spool.tile([TILE, cmax * H], BF16, tag="t1")
            a2b = bass.AP(tensor=c_a2r[:].tensor, offset=c_a2r[:].offset,
                          ap=[c_a2r[:].ap[0], [0, ct], [1, H]])
            nc.vector.tensor_tensor(
                out=t1[:, :ct * H].rearrange("p (i h) -> p i h", i=ct),
                in0=g[:].rearrange("p (i h) -> p i h", i=ct),
                in1=a2b, op=ALU.mult)
            w = small.tile([TILE, cmax], F32, tag="w")
            nc.vector.reduce_sum(
                out=w[:, :ct],
                in_=t1[:, :ct * H].rearrange("p (i h) -> p i h", i=ct),
                axis=mybir.AxisListType.X)
            # u = src . a1 (slot 0), then u' = u + a_b
            t2 = small.tile([TILE, H], BF16, tag="t2")
            nc.vector.tensor_tensor(out=t2[:], in0=g[:, :H], in1=c_a1r[:], op=ALU.mult)
            u = small.tile([TILE, 1], F32, tag="u")
            nc.vector.reduce_sum(
                out=u[:], in_=t2[:].rearrange("p (i h) -> p i h", i=1),
                axis=mybir.AxisListType.X)
            up = small.tile([TILE, 1], F32, tag="up")
            nc.vector.tensor_scalar(
                out=up[:], in0=u[:], scalar1=c_ab2[:, 0:1], scalar2=None,
                op0=ALU.add)
            return w, up

        sc = {0: scores(0)}
        for t in range(nt):
            ct = int(slots[t])
            o0 = int(offs[t])
            g = gtiles[t]
            w, up = sc.pop(t)

            # s = leaky(w + u'), then -1e9 on pad slots
            s0 = small.tile([TILE, cmax], F32, tag="s0")
            nc.scalar.activation(
                out=s0[:, :ct], in_=w[:, :ct], func=AF.Identity, bias=up[:, 0:1])
            s = small.tile([TILE, cmax], F32, tag="s")
            nc.vector.scalar_tensor_tensor(
                out=s[:, :ct], in0=s0[:, :ct], scalar=0.2, in1=s0[:, :ct],
                op0=ALU.mult, op1=ALU.max)
            nc.vector.scalar_tensor_tensor(
                out=s[:, :ct], in0=c_padm[:, o0:o0 + ct], scalar=-1e9,
                in1=s[:, :ct], op0=ALU.mult, op1=ALU.add)
            # softmax
            negm = small.tile([TILE, 1], F32, tag="negm")
            nc.vector.tensor_reduce(
                out=negm[:], in_=s[:, :ct], axis=mybir.AxisListType.X, op=ALU.max,
                negate=True)
            e = small.tile([TILE, cmax], F32, tag="e")
            zsum = small.tile([TILE, 1], F32, tag="zsum")
            nc.scalar.activation(
                out=e[:, :ct], in_=s[:, :ct], func=AF.Exp, bias=negm[:, 0:1],
                accum_out=zsum[:])
            r = small.tile([TILE, 1], F32, tag="r")
            nc.vector.reciprocal(out=r[:], in_=zsum[:])
            att = small.tile([TILE, cmax], F32, tag="att")
            nc.scalar.activation(
                out=att[:, :ct], in_=e[:, :ct], func=AF.Copy, scale=r[:, 0:1])

            # next tile's score pass rides the ACT latency above
            if t + 1 < nt:
                sc[t + 1] = scores(t + 1)

            # weighted sum: gs = g * att (stride-0-inner bcast), add-tree
            gs = spool.tile([TILE, cmax * H], BF16, tag="gs")
            attb = bass.AP(tensor=att[:].tensor, offset=att[:].offset,
                           ap=[att[:].ap[0], [1, ct], [0, H]])
            nc.vector.tensor_tensor(
                out=gs[:, :ct * H].rearrange("p (i h) -> p i h", i=ct),
                in0=attb,
                in1=g[:].rearrange("p (i h) -> p i h", i=ct), op=ALU.mult)
            k = ct
            while k > 2:
                half = k // 2
                nc.vector.tensor_tensor(
                    out=gs[:, :half * H], in0=gs[:, :half * H],
                    in1=gs[:, half * H:2 * half * H], op=ALU.add)
                if k - 2 * half:
                    nc.vector.tensor_tensor(
                        out=gs[:, (half - 1) * H:half * H],
                        in0=gs[:, (half - 1) * H:half * H],
                        in1=gs[:, (k - 1) * H:k * H], op=ALU.add)
                k = half
            v = small.tile([TILE, H], F32, tag="v")
            nc.vector.tensor_tensor(
                out=v[:], in0=gs[:, :H], in1=gs[:, H:2 * H], op=ALU.add)
            vb = small.tile([TILE, H], BF16, tag="vb")
            nc.scalar.activation(out=vb[:], in_=v[:], func=AF.Copy)

            # transpose v via PE identity
            vps = psum.tile([H, TILE], F32, tag="vps")
            nc.tensor.matmul(out=vps[:], lhsT=vb[:], rhs=c_id[:], start=True, stop=True)
            vt = small.tile([H, TILE], BF16, tag="vt")
            nc.scalar.activation(out=vt[:], in_=vps[:], func=AF.Copy)

            # MLP head
            o1p = psum.tile([H, TILE], F32, tag="o1p")
            nc.tensor.matmul(out=o1p[:], lhsT=c_w1a[:], rhs=vt[:], start=True, stop=False)
            nc.tensor.matmul(
                out=o1p[:], lhsT=c_w1b[:], rhs=c_stats[:, t * TILE:(t + 1) * TILE],
                start=False, stop=True)
            o1c = small.tile([H, TILE], BF16, tag="o1c")
            nc.scalar.activation(out=o1c[:], in_=o1p[:], func=AF.Identity, bias=c_b1[:, 0:1])
            o1 = small.tile([H, TILE], BF16, tag="o1")
            nc.vector.scalar_tensor_tensor(
                out=o1[:], in0=o1c[:], scalar=0.2, in1=o1c[:], op0=ALU.mult, op1=ALU.max)
            o2p = psum.tile([TILE, H], F32, tag="o2p")
            nc.tensor.matmul(out=o2p[:], lhsT=o1[:], rhs=c_w2[:], start=True, stop=False)
            nc.tensor.matmul(out=o2p[:], lhsT=c_ones[:], rhs=c_b2[:], start=False, stop=True)
            otc = small.tile([TILE, H], F32, tag="otc")
            nc.scalar.activation(out=otc[:], in_=o2p[:], func=AF.Copy)
            ot = opool.tile([TILE, H], F32, tag="ot")
            nc.vector.scalar_tensor_tensor(
                out=ot[:], in0=otc[:], scalar=0.2, in1=otc[:], op0=ALU.mult, op1=ALU.max)
            nc.sync.dma_start(
                out=bass.AP(tensor=out.ap().tensor, offset=t * TILE * H,
                            ap=[[H, TILE], [1, H]]),
                in_=ot[:])

    nc.finalize()
    # Align queue_num with the Tile-assigned DMASW lane (lane = scheduled
    # Pool-DMA position % 8, queue must be lane % 4 -- the scheduler may
    # reorder, and a DMASW sem is locked to one SWDGE queue). Safe because
    # the idx tiles are replicated across all 128 partitions, so the ucode
    # reads the same indices from any queue's channel group.
    import concourse.bass_isa as bass_isa
    i = 0
    for bb in nc.m.functions[0].blocks:
        for inst in bb.instructions:
            if (inst.engine == mybir.EngineType.Pool
                    and isinstance(inst, bass_isa.AnyDMAInstruction)):
                inst.queue_num = (i % 8) % 4
                i += 1
    return nc


def _prep_inputs(subgraph, neighs, mask, local_stats, global_stats,
                 emb_table, a_w, a_b, fc1_w, fc1_b, fc2_w, fc2_b):
    """Host-side layout/sharding prep.

    Returns (in_maps, orders, key) where key = (slots, ni_hw, v_hw)."""
    bf = ml_dtypes.bfloat16
    a1 = a_w[0, :H]
    a2 = a_w[0, H:]
    shared = {
        "emb": np.ascontiguousarray(emb_table).astype(bf),
        "a2rep": a2.reshape(1, H).astype(bf),
        "a1rep": a1.reshape(1, H).astype(bf),
        "ab_rep": np.broadcast_to(a_b.astype(np.float32), (TILE, 1)).copy(),
        "ident": np.eye(TILE, dtype=np.float32).astype(bf),
        "w1t_a": np.ascontiguousarray(fc1_w[:, :H].T).astype(bf),
        "w1t_b": np.ascontiguousarray(fc1_w[:, H:].T).astype(bf),
        "b1": fc1_b.reshape(H, 1).astype(np.float32),
        "w2t": np.ascontiguousarray(fc2_w.T).astype(bf),
        "b2row": fc2_b.reshape(1, H).astype(bf),
        "onesc": np.ones((1, TILE), dtype=np.float32).astype(bf),
    }
    keep = mask[:, :, :, 0] < 0.5          # [B,S,N] True = neighbor survives
    counts = 1 + keep.sum(axis=2)          # [B,S]
    orders = np.argsort(-counts, axis=1, kind="stable")

    slots = []
    for t in range(NT):
        c = 0
        for b in range(B):
            c = max(c, int(counts[b, orders[b, t * TILE]]))
        slots.append(c)
    slots = tuple(slots)
    offs = np.concatenate([[0], np.cumsum(slots)]).astype(int)
    ctot = int(offs[-1])
    # absolute slot ids per core (-1 = pad)
    idx_abs_all, padm_all = [], []
    uniq_hb = [[None] * B for _ in range(NG)]
    for b in range(B):
        order = orders[b]
        idx_abs = np.full((TILE, ctot), -1, dtype=np.int64)
        padm = np.zeros((TILE, ctot), dtype=np.float32)
        for t in range(NT):
            ct = slots[t]
            o0 = offs[t]
            nodes = order[t * TILE:(t + 1) * TILE]
            idx_abs[:, o0] = subgraph[b, nodes]
            for p in range(TILE):
                n = nodes[p]
                kn = neighs[b, n][keep[b, n]]
                idx_abs[p, o0 + 1:o0 + 1 + len(kn)] = kn
                padm[p, o0 + 1 + len(kn):o0 + ct] = 1.0
        idx_abs_all.append(idx_abs)
        padm_all.append(padm)
        for h, grp in enumerate(GROUPS):
            cols = np.concatenate(
                [np.arange(offs[t], offs[t + 1]) for t in grp])
            hv = idx_abs[:, cols]
            uniq_hb[h][b] = np.unique(hv[hv >= 0])

    # per (group, window) unique counts; SPMD-uniform valid counts
    wstarts = [[np.searchsorted(uniq_hb[h][b], np.arange(NWIN + 1) * WIN)
                for b in range(B)] for h in range(NG)]
    v_hw = tuple(
        tuple(int(max(wstarts[h][b][w + 1] - wstarts[h][b][w] for b in range(B)))
              for w in range(NWIN))
        for h in range(NG))
    ni_hw = tuple(tuple(_rup(v, 16) for v in v_hw[h]) for h in range(NG))
    blocks_hw = [[_rup(ni, TILE) // TILE for ni in ni_hw[h]] for h in range(NG)]
    blk_off_h = [np.concatenate([[0], np.cumsum(blocks_hw[h])]).astype(int)
                 for h in range(NG)]
    blk_tot_h = [int(blk_off_h[h][-1]) for h in range(NG)]
    srows_h = [TILE * bt for bt in blk_tot_h]
    assert sum(srows_h) < 32768, f"scratch rows {sum(srows_h)} exceed int16"

    def wrap16(a):
        return np.ascontiguousarray(a.reshape(-1, 16).T)

    in_maps = []
    for b in range(B):
        idx1_parts = []
        row_of_rank_h = []
        for h in range(NG):
            uniq = uniq_hb[h][b]
            ws = wstarts[h][b]
            row_of_rank = np.empty(len(uniq), dtype=np.int64)
            for w in range(NWIN):
                if ni_hw[h][w] == 0:
                    continue
                rel = uniq[ws[w]:ws[w + 1]] - w * WIN
                nwb = len(rel)
                a = np.full(ni_hw[h][w], -1, dtype=np.int16)
                a[:nwb] = rel.astype(np.int16)
                a[nwb:v_hw[h][w]] = 0                 # top-up (dup row) for SPMD
                idx1_parts.append(wrap16(a))
                j = np.arange(nwb)
                row_of_rank[ws[w]:ws[w + 1]] = (
                    (j % TILE) * blk_tot_h[h] + blk_off_h[h][w] + j // TILE)
            row_of_rank_h.append(row_of_rank)
        idx1 = np.tile(np.concatenate(idx1_parts, axis=1), (8, 1))

        # phase-2 positional rank lists (slot-major per tile, half-relative)
        idx_abs = idx_abs_all[b]
        idx2_parts = []
        t2h = {t: h for h, grp in enumerate(GROUPS) for t in grp}
        for t in range(NT):
            h = t2h[t]
            uniq = uniq_hb[h][b]
            rr = row_of_rank_h[h]
            ct = slots[t]
            o0 = offs[t]
            blkv = idx_abs[:, o0:o0 + ct]
            ranks = np.searchsorted(uniq, np.clip(blkv, 0, None))
            row2 = rr[ranks]
            row2[blkv < 0] = rr[0]
            assert row2.max() < srows_h[h]
            arr = np.ascontiguousarray(row2.T).ravel()   # j = i*128 + p
            idx2_parts.append(wrap16(arr.astype(np.int16)))
        idx2 = np.tile(np.concatenate(idx2_parts, axis=1), (8, 1))

        order = orders[b]
        st = np.concatenate(
            [local_stats[b][order].T,
             np.broadcast_to(global_stats[b].reshape(1, 1), (1, S))], axis=0)
        m = dict(shared)
        m.update({
            "idx1": idx1, "idx2": idx2, "padm": padm_all[b],
            "statst": np.ascontiguousarray(st).astype(bf),
        })
        in_maps.append(m)
    return in_maps, orders, (slots, ni_hw, v_hw)


last_exec_ns = None
last_results = None


def kernel(**inputs) -> np.ndarray:
    global last_exec_ns, last_results
    in_maps, orders, key = _prep_inputs(**inputs)
    if key not in _cached:
        _cached[key] = _build_program(*key)
    nc = _cached[key]
    trace = bool(int(os.environ.get("KERNEL_TRACE", "0")))
    res = bass_utils.run_bass_kernel_spmd(
        nc, in_maps, core_ids=list(range(B)), trace=trace)
    last_exec_ns = res.exec_time_ns
    last_results = res
    out = np.empty((B, S, H), dtype=np.float32)
    for b in range(B):
        out[b, orders[b]] = res.results[b]["out"]
    return out


if __name__ == "__main__":
    _build_program(
        (33,) * NT,
        ((3504, 3504, 3504, 208), (3504, 3504, 3504, 208)),
        ((3500, 3500, 3500, 200), (3500, 3500, 3500, 200)))
    print("program builds OK")


# revision 35
# speedup vs baseline: 1.0423x; 1.0423x over previous
"""GAT NodeEncoder kernel for Trainium2 (8 NeuronCores, data-parallel over batch).

Reference computation (per batch element b, per node n):
    src  = E[subgraph[b,n]];  nei_i = E[neighs[b,n,i]]
    s_0  = leaky(src@a1 + src@a2 + a_b); s_i = leaky(src@a1 + nei_i@a2 + a_b) + mask_i*-1e9
    att  = softmax(s); v = sum_i att_i * emb_i
    x = leaky(fc1 @ [v; local_stats; gstat] + b1); out = leaky(fc2 @ x + b2)

Sharding: batch B=8 over 8 cores (1 batch row / core), emb table replicated
(uploaded bf16 -- matches the bf16 in-flight compute precision).

Gather strategy. SWDGE descriptor generation costs ~8ns per gathered row on
one Q7 pair, but the 4 SWDGE queues run on different Q7 pairs and overlap
when calls are issued back-to-back. Two-phase scheme, split into halves so
compute starts early:
  Phase 1 (per tile-group of 4): gather the group's UNIQUE table rows with
  dma_gather calls per 32768-row window (int16 idx limit), chunked
  round-robin over the 4 queues; flush each chunk to a DRAM scratch region
  addressable by int16 rank.
  Phase 2 (per group): per 128-node tile, 4 dma_gather sub-calls from the
  group's scratch fetch all (node, slot) rows positionally (slot-major).
  Queue numbers are rewritten post-schedule to match the Tile-assigned
  DMASW semaphore lanes (lane %% 4).

Compute per tile (slots uniform per tile via degree-sorting, masked
neighbors dropped exactly):
  scores   = reduce_X(g * a2_bcast) on DVE; s = leaky(w + u + ab) ACT+DVE
  softmax  = max / exp+accum / recip; att = Copy(e, scale=1/z) on ACT
  weighted = g * att (stride-0-inner broadcast) + halving add-tree on DVE
  head     = PE transpose via identity, fc1/fc2 on PE, bias on ACT
Output rows are stored directly (HWDGE) in sorted order; host unsorts.
"""

import os
from contextlib import ExitStack

import numpy as np
import ml_dtypes

import concourse.bass as bass
import concourse.bacc as bacc
import concourse.tile as tile
from concourse import mybir
from concourse import bass_utils
from concourse import library_config

B, S, N, H, NLS = 8, 1024, 32, 128, 4
NUM_NODES = 100001
TILE = 128
NT = S // TILE
WIN = 32768
NWIN = 4
GROUPS = ((0, 1, 2, 3, 4), (5, 6, 7))      # tile groups (phase-1 split)
NG = len(GROUPS)
F32 = mybir.dt.float32
BF16 = mybir.dt.bfloat16
I32 = mybir.dt.int32
I16 = mybir.dt.int16
AF = mybir.ActivationFunctionType
ALU = mybir.AluOpType

_cached = {}


def _rup(x, m):
    return (x + m - 1) // m * m


def _win_chunks(niw):
    """Split a window's padded index list into <=4 chunks at 128-pos bounds."""
    if niw < 1024:
        return [(0, niw)]
    nch = 8 if niw >= 4096 else 4
    q = _rup((niw + nch - 1) // nch, TILE)
    return [(p0, min(q, niw - p0)) for p0 in range(0, niw, q)]


def _build_program(slots, ni_hw, v_hw):
    """slots: per-tile slot counts (len 8); ni_hw/v_hw: per-GROUP tuples of
    per-window phase-1 static num_idxs / valid counts (SPMD-uniform)."""
    nt = len(slots)
    ctot = int(sum(slots))
    offs = np.concatenate([[0], np.cumsum(slots)]).astype(int)
    cmax = int(max(slots))

    # per-group scratch geometry
    blocks_hw = [[_rup(ni, TILE) // TILE for ni in ni_hw[h]] for h in range(NG)]
    blk_off_h = [np.concatenate([[0], np.cumsum(blocks_hw[h])]).astype(int)
                 for h in range(NG)]
    blk_tot_h = [int(blk_off_h[h][-1]) for h in range(NG)]
    srows_h = [TILE * bt for bt in blk_tot_h]
    sbase_h = np.concatenate([[0], np.cumsum(srows_h)]).astype(int)
    srows = int(sum(srows_h))
    # phase-1 idx columns per (group, window)
    n1cols_h = [[ni // 16 for ni in ni_hw[h]] for h in range(NG)]
    c1off_h = [np.concatenate([[0], np.cumsum(n1cols_h[h])]).astype(int)
               for h in range(NG)]
    c1base = np.concatenate([[0], np.cumsum([int(c1off_h[h][-1]) for h in range(NG)])]).astype(int)
    c1tot = int(c1base[-1])
    # phase-2 idx columns per tile
    n2cols = [TILE * int(c) // 16 for c in slots]
    c2off = np.concatenate([[0], np.cumsum(n2cols)]).astype(int)

    nc = bacc.Bacc(target_bir_lowering=False, debug=False, enable_asserts=False,
                   num_swdge_queues=4)

    emb = nc.dram_tensor("emb", [NUM_NODES, H], BF16, kind="ExternalInput")
    idx1 = nc.dram_tensor("idx1", [TILE, c1tot], I16, kind="ExternalInput")
    idx2 = nc.dram_tensor("idx2", [TILE, int(c2off[-1])], I16, kind="ExternalInput")
    padm = nc.dram_tensor("padm", [TILE, ctot], F32, kind="ExternalInput")
    statst = nc.dram_tensor("statst", [NLS + 1, S], BF16, kind="ExternalInput")
    a2rep_d = nc.dram_tensor("a2rep", [1, H], BF16, kind="ExternalInput")
    a1rep_d = nc.dram_tensor("a1rep", [1, H], BF16, kind="ExternalInput")
    ab_rep = nc.dram_tensor("ab_rep", [TILE, 1], F32, kind="ExternalInput")
    ident = nc.dram_tensor("ident", [TILE, TILE], BF16, kind="ExternalInput")
    w1t_a = nc.dram_tensor("w1t_a", [H, H], BF16, kind="ExternalInput")
    w1t_b = nc.dram_tensor("w1t_b", [NLS + 1, H], BF16, kind="ExternalInput")
    b1 = nc.dram_tensor("b1", [H, 1], F32, kind="ExternalInput")
    w2t = nc.dram_tensor("w2t", [H, H], BF16, kind="ExternalInput")
    b2row = nc.dram_tensor("b2row", [1, H], BF16, kind="ExternalInput")
    onesc = nc.dram_tensor("onesc", [1, TILE], BF16, kind="ExternalInput")
    out = nc.dram_tensor("out", [S, H], F32, kind="ExternalOutput")

    with tile.TileContext(nc) as tc, ExitStack() as ctx:
        dpool = ctx.enter_context(tc.tile_pool(name="dram", bufs=1, space="DRAM"))
        const = ctx.enter_context(tc.tile_pool(name="const", bufs=1))
        psum = ctx.enter_context(tc.tile_pool(name="psum", bufs=2, space="PSUM"))

        scratch = dpool.tile([srows, H], BF16)

        nc.gpsimd.load_library(library_config.mlp)

        # ---- constants: idx1 first (gates phase-1), all on the sync queue
        # (idle until flushes) so the ACT engine is free for early compute ----
        p1pool = ctx.enter_context(tc.tile_pool(name="p1", bufs=1))
        c_idx1 = p1pool.tile([TILE, c1tot], I16)
        nc.sync.dma_start(out=c_idx1[:], in_=idx1[:, :])
        c_idx2_0 = const.tile([TILE, int(c2off[-1])], I16)
        nc.sync.dma_start(out=c_idx2_0[:], in_=idx2[:, :])
        c_padm0 = const.tile([TILE, ctot], F32)
        nc.scalar.dma_start(out=c_padm0[:], in_=padm[:, :])
        c_stats = const.tile([NLS + 1, S], BF16)
        nc.scalar.dma_start(out=c_stats[:], in_=statst[:, :])
        c_ab = const.tile([TILE, 1], F32)
        nc.scalar.dma_start(out=c_ab[:], in_=ab_rep[:, :])
        c_id = const.tile([TILE, TILE], BF16)
        nc.scalar.dma_start(out=c_id[:], in_=ident[:, :])
        c_w1a = const.tile([H, H], BF16)
        nc.scalar.dma_start(out=c_w1a[:], in_=w1t_a[:, :])
        c_w1b = const.tile([NLS + 1, H], BF16)
        nc.scalar.dma_start(out=c_w1b[:], in_=w1t_b[:, :])
        c_b1 = const.tile([H, 1], F32)
        nc.scalar.dma_start(out=c_b1[:], in_=b1[:, :])
        c_w2 = const.tile([H, H], BF16)
        nc.scalar.dma_start(out=c_w2[:], in_=w2t[:, :])
        c_b2 = const.tile([1, H], BF16)
        nc.scalar.dma_start(out=c_b2[:], in_=b2row[:, :])
        c_ones = const.tile([1, TILE], BF16)
        nc.scalar.dma_start(out=c_ones[:], in_=onesc[:, :])
        # a1/a2 rows replicated to 128 partitions via PE ones-trick (keeps the
        # Pool engine free of non-gather DMAs so DMASW lanes map 1:1 to queues)
        c_a2row = const.tile([1, H], BF16)
        nc.scalar.dma_start(out=c_a2row[:], in_=a2rep_d[:, :])
        c_a1row = const.tile([1, H], BF16)
        nc.scalar.dma_start(out=c_a1row[:], in_=a1rep_d[:, :])

        # ---- fences: absorb const-DMA sems onto consuming engines ----
        c_idx2 = const.tile([TILE, int(c2off[-1])], I16)
        nc.vector.tensor_copy(out=c_idx2[:], in_=c_idx2_0[:])
        reps = psum.tile([TILE, H], F32, tag="dfence")
        c_a2r = const.tile([TILE, H], BF16)
        nc.tensor.matmul(out=reps[:], lhsT=c_ones[:], rhs=c_a2row[:], start=True, stop=True)
        nc.scalar.activation(out=c_a2r[:], in_=reps[:], func=AF.Copy)
        reps2 = psum.tile([TILE, H], F32, tag="dfence")
        c_a1r = const.tile([TILE, H], BF16)
        nc.tensor.matmul(out=reps2[:], lhsT=c_ones[:], rhs=c_a1row[:], start=True, stop=True)
        nc.scalar.activation(out=c_a1r[:], in_=reps2[:], func=AF.Copy)
        c_padm = const.tile([TILE, ctot], F32)
        nc.vector.tensor_copy(out=c_padm[:], in_=c_padm0[:])
        c_ab2 = const.tile([TILE, 1], F32)
        nc.vector.tensor_copy(out=c_ab2[:], in_=c_ab[:])
        dpsum = psum.tile([TILE, TILE], F32, tag="dfence")
        nc.tensor.matmul(out=dpsum[:], lhsT=c_id[:], rhs=c_w1a[:], start=True, stop=True)
        nc.tensor.matmul(out=dpsum[:], lhsT=c_w2[:], rhs=c_id[:], start=True, stop=True)
        nc.tensor.matmul(
            out=dpsum[:], lhsT=c_w1b[:], rhs=c_stats[:, 0:TILE], start=True, stop=True)
        nc.tensor.matmul(out=dpsum[:], lhsT=c_ones[:], rhs=c_b2[:], start=True, stop=True)
        dact = const.tile([TILE, 1], F32)
        nc.scalar.activation(out=dact[:], in_=c_ab2[:], func=AF.Identity, bias=c_b1[:, 0:1])

        gpool = ctx.enter_context(tc.tile_pool(name="gpool", bufs=1))
        spool = ctx.enter_context(tc.tile_pool(name="spool", bufs=3))
        small = ctx.enter_context(tc.tile_pool(name="small", bufs=6))
        opool = ctx.enter_context(tc.tile_pool(name="opool", bufs=2))

        qrr = 0
        gtiles = {}

        def phase1(h):
            nonlocal qrr
            g1 = p1pool.tile([TILE, blk_tot_h[h] * H], BF16, tag=f"g1{h}")
            blk_off = blk_off_h[h]
            for w in range(NWIN):
                niw, vw = int(ni_hw[h][w]), int(v_hw[h][w])
                if niw == 0:
                    continue
                span = min(WIN, NUM_NODES - w * WIN)
                src_ap = bass.AP(tensor=emb.ap().tensor, offset=w * WIN * H,
                                 ap=[[H, span], [1, H]])
                for (p0, ln) in _win_chunks(niw):
                    vc = max(0, min(ln, vw - p0))
                    b0 = int(blk_off[w]) + p0 // TILE
                    nblk = _rup(ln, TILE) // TILE
                    nc.gpsimd.dma_gather(
                        g1[:, b0 * H:(b0 + nblk) * H].rearrange(
                            "p (b h) -> p b h", b=nblk),
                        src_ap,
                        c_idx1[:, int(c1base[h]) + int(c1off_h[h][w]) + p0 // 16:
                               int(c1base[h]) + int(c1off_h[h][w]) + (p0 + ln) // 16],
                        ln, vc, H,
                        single_packet=False, queue_num=qrr % 4)
                    qrr += 1
                    # flush chunk to scratch rows sbase + p*blk_tot + b0 + u
                    nc.sync.dma_start(
                        out=bass.AP(
                            tensor=scratch[:].tensor,
                            offset=scratch[:].offset + (int(sbase_h[h]) + b0) * H,
                            ap=[[blk_tot_h[h] * H, TILE], [H, nblk], [1, H]]),
                        in_=g1[:, b0 * H:(b0 + nblk) * H])

        def phase2(h):
            nonlocal qrr
            for t in GROUPS[h]:
                ct = int(slots[t])
                g = gpool.tile([TILE, ct * H], BF16, tag=f"g{t}")
                src = bass.AP(tensor=scratch[:].tensor,
                              offset=scratch[:].offset + int(sbase_h[h]) * H,
                              ap=[[H, srows_h[h]], [1, H]])
                cq = (ct + 3) // 4
                bounds = list(range(0, ct, cq)) + [ct]
                for (s0_, s1_) in zip(bounds[:-1], bounds[1:]):
                    nidx = TILE * (s1_ - s0_)
                    nc.gpsimd.dma_gather(
                        g[:, s0_ * H:s1_ * H].rearrange(
                            "p (i h) -> p i h", i=s1_ - s0_),
                        src,
                        c_idx2[:, int(c2off[t]) + s0_ * 8:
                               int(c2off[t]) + s1_ * 8],
                        nidx, nidx, H,
                        single_packet=False, queue_num=qrr % 4)
                    qrr += 1
                gtiles[t] = g

        for h in range(NG):
            phase1(h)
            phase2(h)

        # ---- per-tile compute (software-pipelined: tile t+1's score pass is
        # emitted between tile t's softmax and weighted-sum so DVE has work
        # while ACT runs) ----
        def scores(t):
            ct = int(slots[t])
            g = gtiles[t]
            t1 = spool.tile([TILE, cmax * H], BF16, tag="t1")
            a2b = bass.AP(tensor=c_a2r[:].tensor, offset=c_a2r[:].offset,
                          ap=[c_a2r[:].ap[0], [0, ct], [1, H]])
            nc.vector.tensor_tensor(
                out=t1[:, :ct * H].rearrange("p (i h) -> p i h", i=ct),
                in0=g[:].rearrange("p (i h) -> p i h", i=ct),
                in1=a2b, op=ALU.mult)
            w = small.tile([TILE, cmax], F32, tag="w")
            nc.vector.reduce_sum(
                out=w[:, :ct],
                in_=t1[:, :ct * H].rearrange("p (i h) -> p i h", i=ct),
                axis=mybir.AxisListType.X)
            # u = src . a1 (slot 0), then u' = u + a_b
            t2 = small.tile([TILE, H], BF16, tag="t2")
            nc.vector.tensor_tensor(out=t2[:], in0=g[:, :H], in1=c_a1r[:], op=ALU.mult)
            u = small.tile([TILE, 1], F32, tag="u")
            nc.vector.reduce_sum(
                out=u[:], in_=t2[:].rearrange("p (i h) -> p i h", i=1),
                axis=mybir.AxisListType.X)
            up = small.tile([TILE, 1], F32, tag="up")
            nc.vector.tensor_scalar(
                out=up[:], in0=u[:], scalar1=c_ab2[:, 0:1], scalar2=None,
                op0=ALU.add)
            return w, up

        sc = {0: scores(0)}
        for t in range(nt):
            ct = int(slots[t])
            o0 = int(offs[t])
            g = gtiles[t]
            w, up = sc.pop(t)

            # s = leaky(w + u'), then -1e9 on pad slots
            s0 = small.tile([TILE, cmax], F32, tag="s0")
            nc.scalar.activation(
                out=s0[:, :ct], in_=w[:, :ct], func=AF.Identity, bias=up[:, 0:1])
            s = small.tile([TILE, cmax], F32, tag="s")
            nc.vector.scalar_tensor_tensor(
                out=s[:, :ct], in0=s0[:, :ct], scalar=0.2, in1=s0[:, :ct],
                op0=ALU.mult, op1=ALU.max)
            nc.vector.scalar_tensor_tensor(
                out=s[:, :ct], in0=c_padm[:, o0:o0 + ct], scalar=-1e9,
                in1=s[:, :ct], op0=ALU.mult, op1=ALU.add)
            # softmax
            negm = small.tile([TILE, 1], F32, tag="negm")
            nc.vector.tensor_reduce(
                out=negm[:], in_=s[:, :ct], axis=mybir.AxisListType.X, op=ALU.max,
                negate=True)
            e = small.tile([TILE, cmax], F32, tag="e")
            zsum = small.tile([TILE, 1], F32, tag="zsum")
            nc.scalar.activation(
                out=e[:, :ct], in_=s[:, :ct], func=AF.Exp, bias=negm[:, 0:1],
                accum_out=zsum[:])
            r = small.tile([TILE, 1], F32, tag="r")
            nc.vector.reciprocal(out=r[:], in_=zsum[:])
            att = small.tile([TILE, cmax], F32, tag="att")
            nc.scalar.activation(
                out=att[:, :ct], in_=e[:, :ct], func=AF.Copy, scale=r[:, 0:1])

            # next tile's score pass rides the ACT latency above
            if t + 1 < nt:
                sc[t + 1] = scores(t + 1)

            # weighted sum: gs = g * att (stride-0-inner bcast), add-tree
            gs = spool.tile([TILE, cmax * H], BF16, tag="gs")
            attb = bass.AP(tensor=att[:].tensor, offset=att[:].offset,
                           ap=[att[:].ap[0], [1, ct], [0, H]])
            nc.vector.tensor_tensor(
                out=gs[:, :ct * H].rearrange("p (i h) -> p i h", i=ct),
                in0=attb,
                in1=g[:].rearrange("p (i h) -> p i h", i=ct), op=ALU.mult)
            k = ct
            while k > 2:
                half = k // 2
                nc.vector.tensor_tensor(
                    out=gs[:, :half * H], in0=gs[:, :half * H],
                    in1=gs[:, half * H:2 * half * H], op=ALU.add)
                if k - 2 * half:
                    nc.vector.tensor_tensor(
                        out=gs[:, (half - 1) * H:half * H],
                        in0=gs[:, (half - 1) * H:half * H],
                        in1=gs[:, (k - 1) * H:k * H], op=ALU.add)
                k = half
            v = small.tile([TILE, H], F32, tag="v")
            nc.vector.tensor_tensor(
                out=v[:], in0=gs[:, :H], in1=gs[:, H:2 * H], op=ALU.add)
            vb = small.tile([TILE, H], BF16, tag="vb")
            nc.scalar.activation(out=vb[:], in_=v[:], func=AF.Copy)

            # transpose v via PE identity
            vps = psum.tile([H, TILE], F32, tag="vps")
            nc.tensor.matmul(out=vps[:], lhsT=vb[:], rhs=c_id[:], start=True, stop=True)
            vt = small.tile([H, TILE], BF16, tag="vt")
            nc.scalar.activation(out=vt[:], in_=vps[:], func=AF.Copy)

            # MLP head
            o1p = psum.tile([H, TILE], F32, tag="o1p")
            nc.tensor.matmul(out=o1p[:], lhsT=c_w1a[:], rhs=vt[:], start=True, stop=False)
            nc.tensor.matmul(
                out=o1p[:], lhsT=c_w1b[:], rhs=c_stats[:, t * TILE:(t + 1) * TILE],
                start=False, stop=True)
            o1c = small.tile([H, TILE], BF16, tag="o1c")
            nc.scalar.activation(out=o1c[:], in_=o1p[:], func=AF.Identity, bias=c_b1[:, 0:1])
            o1 = small.tile([H, TILE], BF16, tag="o1")
            nc.vector.scalar_tensor_tensor(
                out=o1[:], in0=o1c[:], scalar=0.2, in1=o1c[:], op0=ALU.mult, op1=ALU.max)
            o2p = psum.tile([TILE, H], F32, tag="o2p")
            nc.tensor.matmul(out=o2p[:], lhsT=o1[:], rhs=c_w2[:], start=True, stop=False)
            nc.tensor.matmul(out=o2p[:], lhsT=c_ones[:], rhs=c_b2[:], start=False, stop=True)
            otc = small.tile([TILE, H], F32, tag="otc")
            nc.scalar.activation(out=otc[:], in_=o2p[:], func=AF.Copy)
            ot = opool.tile([TILE, H], F32, tag="ot")
            nc.vector.scalar_tensor_tensor(
                out=ot[:], in0=otc[:], scalar=0.2, in1=otc[:], op0=ALU.mult, op1=ALU.max)
            nc.sync.dma_start(
                out=bass.AP(tensor=out.ap().tensor, offset=t * TILE * H,
                            ap=[[H, TILE], [1, H]]),
                in_=ot[:])

    nc.finalize()
    # Align queue_num with the Tile-assigned DMASW lane (lane = scheduled
    # Pool-DMA position % 8, queue must be lane % 4 -- the scheduler may
    # reorder, and a DMASW sem is locked to one SWDGE queue). Safe because
    # the idx tiles are replicated across all 128 partitions, so the ucode
    # reads the same indices from any queue's channel group.
    import concourse.bass_isa as bass_isa
    i = 0
    for bb in nc.m.functions[0].blocks:
        for inst in bb.instructions:
            if (inst.engine == mybir.EngineType.Pool
                    and isinstance(inst, bass_isa.AnyDMAInstruction)):
                inst.queue_num = (i % 8) % 4
                i += 1
    return nc


def _prep_inputs(subgraph, neighs, mask, local_stats, global_stats,
                 emb_table, a_w, a_b, fc1_w, fc1_b, fc2_w, fc2_b):
    """Host-side layout/sharding prep.

    Returns (in_maps, orders, key) where key = (slots, ni_hw, v_hw)."""
    bf = ml_dtypes.bfloat16
    a1 = a_w[0, :H]
    a2 = a_w[0, H:]
    shared = {
        "emb": np.ascontiguousarray(emb_table).astype(bf),
        "a2rep": a2.reshape(1, H).astype(bf),
        "a1rep": a1.reshape(1, H).astype(bf),
        "ab_rep": np.broadcast_to(a_b.astype(np.float32), (TILE, 1)).copy(),
        "ident": np.eye(TILE, dtype=np.float32).astype(bf),
        "w1t_a": np.ascontiguousarray(fc1_w[:, :H].T).astype(bf),
        "w1t_b": np.ascontiguousarray(fc1_w[:, H:].T).astype(bf),
        "b1": fc1_b.reshape(H, 1).astype(np.float32),
        "w2t": np.ascontiguousarray(fc2_w.T).astype(bf),
        "b2row": fc2_b.reshape(1, H).astype(bf),
        "onesc": np.ones((1, TILE), dtype=np.float32).astype(bf),
    }
    keep = mask[:, :, :, 0] < 0.5          # [B,S,N] True = neighbor survives
    counts = 1 + keep.sum(axis=2)          # [B,S]
    orders = np.argsort(-counts, axis=1, kind="stable")

    slots = []
    for t in range(NT):
        c = 0
        for b in range(B):
            c = max(c, int(counts[b, orders[b, t * TILE]]))
        slots.append(c)
    slots = tuple(slots)
    offs = np.concatenate([[0], np.cumsum(slots)]).astype(int)
    ctot = int(offs[-1])
    # absolute slot ids per core (-1 = pad)
    idx_abs_all, padm_all = [], []
    uniq_hb = [[None] * B for _ in range(NG)]
    for b in range(B):
        order = orders[b]
        idx_abs = np.full((TILE, ctot), -1, dtype=np.int64)
        padm = np.zeros((TILE, ctot), dtype=np.float32)
        for t in range(NT):
            ct = slots[t]
            o0 = offs[t]
            nodes = order[t * TILE:(t + 1) * TILE]
            idx_abs[:, o0] = subgraph[b, nodes]
            for p in range(TILE):
                n = nodes[p]
                kn = neighs[b, n][keep[b, n]]
                idx_abs[p, o0 + 1:o0 + 1 + len(kn)] = kn
                padm[p, o0 + 1 + len(kn):o0 + ct] = 1.0
        idx_abs_all.append(idx_abs)
        padm_all.append(padm)
        for h, grp in enumerate(GROUPS):
            cols = np.concatenate(
                [np.arange(offs[t], offs[t + 1]) for t in grp])
            hv = idx_abs[:, cols]
            uniq_hb[h][b] = np.unique(hv[hv >= 0])

    # per (group, window) unique counts; SPMD-uniform valid counts
    wstarts = [[np.searchsorted(uniq_hb[h][b], np.arange(NWIN + 1) * WIN)
                for b in range(B)] for h in range(NG)]
    v_hw = tuple(
        tuple(int(max(wstarts[h][b][w + 1] - wstarts[h][b][w] for b in range(B)))
              for w in range(NWIN))
        for h in range(NG))
    ni_hw = tuple(tuple(_rup(v, 16) for v in v_hw[h]) for h in range(NG))
    blocks_hw = [[_rup(ni, TILE) // TILE for ni in ni_hw[h]] for h in range(NG)]
    blk_off_h = [np.concatenate([[0], np.cumsum(blocks_hw[h])]).astype(int)
                 for h in range(NG)]
    blk_tot_h = [int(blk_off_h[h][-1]) for h in range(NG)]
    srows_h = [TILE * bt for bt in blk_tot_h]
    assert sum(srows_h) < 32768, f"scratch rows {sum(srows_h)} exceed int16"

    def wrap16(a):
        return np.ascontiguousarray(a.reshape(-1, 16).T)

    in_maps = []
    for b in range(B):
        idx1_parts = []
        row_of_rank_h = []
        for h in range(NG):
            uniq = uniq_hb[h][b]
            ws = wstarts[h][b]
            row_of_rank = np.empty(len(uniq), dtype=np.int64)
            for w in range(NWIN):
                if ni_hw[h][w] == 0:
                    continue
                rel = uniq[ws[w]:ws[w + 1]] - w * WIN
                nwb = len(rel)
                a = np.full(ni_hw[h][w], -1, dtype=np.int16)
                a[:nwb] = rel.astype(np.int16)
                a[nwb:v_hw[h][w]] = 0                 # top-up (dup row) for SPMD
                idx1_parts.append(wrap16(a))
                j = np.arange(nwb)
                row_of_rank[ws[w]:ws[w + 1]] = (
                    (j % TILE) * blk_tot_h[h] + blk_off_h[h][w] + j // TILE)
            row_of_rank_h.append(row_of_rank)
        idx1 = np.tile(np.concatenate(idx1_parts, axis=1), (8, 1))

        # phase-2 positional rank lists (slot-major per tile, half-relative)
        idx_abs = idx_abs_all[b]
        idx2_parts = []
        t2h = {t: h for h, grp in enumerate(GROUPS) for t in grp}
        for t in range(NT):
            h = t2h[t]
            uniq = uniq_hb[h][b]
            rr = row_of_rank_h[h]
            ct = slots[t]
            o0 = offs[t]
            blkv = idx_abs[:, o0:o0 + ct]
            ranks = np.searchsorted(uniq, np.clip(blkv, 0, None))
            row2 = rr[ranks]
            row2[blkv < 0] = rr[0]
            assert row2.max() < srows_h[h]
            arr = np.ascontiguousarray(row2.T).ravel()   # j = i*128 + p
            idx2_parts.append(wrap16(arr.astype(np.int16)))
        idx2 = np.tile(np.concatenate(idx2_parts, axis=1), (8, 1))

        order = orders[b]
        st = np.concatenate(
            [local_stats[b][order].T,
             np.broadcast_to(global_stats[b].reshape(1, 1), (1, S))], axis=0)
        m = dict(shared)
        m.update({
            "idx1": idx1, "idx2": idx2, "padm": padm_all[b],
            "statst": np.ascontiguousarray(st).astype(bf),
        })
        in_maps.append(m)
    return in_maps, orders, (slots, ni_hw, v_hw)


last_exec_ns = None
last_results = None


def kernel(**inputs) -> np.ndarray:
    global last_exec_ns, last_results
    in_maps, orders, key = _prep_inputs(**inputs)
    if key not in _cached:
        _cached[key] = _build_program(*key)
    nc = _cached[key]
    trace = bool(int(os.environ.get("KERNEL_TRACE", "0")))
    res = bass_utils.run_bass_kernel_spmd(
        nc, in_maps, core_ids=list(range(B)), trace=trace)
    last_exec_ns = res.exec_time_ns
    last_results = res
    out = np.empty((B, S, H), dtype=np.float32)
    for b in range(B):
        out[b, orders[b]] = res.results[b]["out"]
    return out


if __name__ == "__main__":
    _build_program(
        (33,) * NT,
        ((3504, 3504, 3504, 208), (3504, 3504, 3504, 208)),
        ((3500, 3500, 3500, 200), (3500, 3500, 3500, 200)))
    print("program builds OK")
